# revision 1
# baseline (speedup 1.0000x reference)
"""MoE FFN (16 experts, top-2, SwiGLU, + shared expert) on 8 trn2 NeuronCores.

Strategy (expert-parallel, per sharding hint):
  - Host computes the (tiny) router in fp64, dispatches tokens by topk_idx:
    each core c owns experts {2c, 2c+1} and receives its experts' tokens
    gathered + transposed into [feature, token] layout, capacity-padded to C.
  - Device runs the heavy compute: per expert  gate/up projections (fp32r
    matmuls at full PE rate), SwiGLU, down projection, scaled by the top-2
    softmax combine weight.
  - Shared expert is token-parallel: core c processes tokens [512c, 512c+512)
    with the full (replicated) shared weights.
  - Host scatter-adds per-expert outputs back by token index (the "unshard")
    and adds the shared-expert shard outputs. No on-device collectives.

All device matmuls contract over the feature dim, which lives on SBUF
partitions; activations stay in [feature, token] layout so no on-device
transposes are needed (host pre-transposes the gathered tokens / weights).
"""

import math
import os
import sys

for _p in ("/opt/trn_rl_repo", "/root/.axon_site", "/root/.axon_site/_ro/trn_rl_repo",
           "/root/.axon_site/_ro/pypackages"):
    if os.path.isdir(_p) and _p not in sys.path:
        sys.path.append(_p)

import numpy as np

# The agent image's `antenv` package lacks `axon_hooks`, which
# concourse.bass_utils imports when BASS_TRACE=1. Install a compatible
# shim (and register the real NTFF hook if the axon .so is present) so
# tracing works and trace=True doesn't crash.
try:
    from antenv import axon_hooks as _ah  # noqa: F401
except ImportError:
    try:
        import types

        import antenv as _antenv

        _ah = types.ModuleType("antenv.axon_hooks")
        _ah._hook = None
        _ah.set_axon_ntff_profile_hook = lambda h: setattr(_ah, "_hook", h)
        _ah.get_axon_ntff_profile_hook = lambda: _ah._hook
        sys.modules["antenv.axon_hooks"] = _ah
        _antenv.axon_hooks = _ah
        try:
            from trn_agent_boot.trn_boot import _ntff_profile_via_ctypes

            if os.path.exists("/opt/axon/libaxon_pjrt.so"):
                _ah._hook = _ntff_profile_via_ctypes("/opt/axon/libaxon_pjrt.so")
        except Exception:
            pass
    except Exception:
        pass

DIM = 1024
ED = 512          # expert hidden dim
E = 16            # experts
TOPK = 2
SH = 1024         # shared expert hidden dim
N_CORES = 8
EXP_PER_CORE = E // N_CORES   # 2
P = 128

USE_BF16 = os.environ.get("MOE_BF16", "1") == "1"
if USE_BF16:
    import ml_dtypes
    _MM_NP_DT = ml_dtypes.bfloat16
else:
    _MM_NP_DT = np.float32

# compiled-program cache keyed by capacity C
_PROGRAMS = {}
LAST_RESULT = None  # BassKernelResults of the most recent run (for test.py)


def _token_tiles(C):
    """Split C into matmul free-dim tiles: <=512 (one PSUM bank), and for
    fp32r mode >=256 (full-rate floor; bf16 has no such constraint)."""
    import math as _m
    n_t = _m.ceil(C / 512)
    base, rem = divmod(C, n_t)
    sizes = [base + (1 if i < rem else 0) for i in range(n_t)]
    tiles, off = [], 0
    for sz in sizes:
        assert sz <= 512 and (USE_BF16 or sz >= 256), (C, sizes)
        tiles.append((off, sz))
        off += sz
    return tiles


def _build_program(C0, C1):
    import concourse.bacc as bacc
    import concourse.mybir as mybir
    import concourse.tile as tile

    f32 = mybir.dt.float32
    f32r = mybir.dt.bfloat16 if USE_BF16 else mybir.dt.float32r
    SIG = mybir.ActivationFunctionType.Silu

    nc = bacc.Bacc("TRN2", target_bir_lowering=False, debug=False)

    CS = [C0, C1]
    # ---- I/O (per-core) ----
    # gathered tokens, transposed: xg{s}[p, dd, t] = x[idx_e[t], dd*128+p]
    xg_ds = [nc.dram_tensor(f"xg{i}", [P, DIM // P, CS[i]], f32r,
                            kind="ExternalInput") for i in range(EXP_PER_CORE)]
    # combine weights pre-broadcast over partitions: bc{s}[p, t] = w_e[t]
    bc_ds = [nc.dram_tensor(f"bc{i}", [P, CS[i]], f32, kind="ExternalInput")
             for i in range(EXP_PER_CORE)]
    # gate/up weights: wgu[e, g_or_u, p, hh, dd, c] = W[e][dd*128+p, hh*128+c]
    wgu_d = nc.dram_tensor("wgu", [EXP_PER_CORE, 2, P, ED // P, DIM // P, P], f32r,
                           kind="ExternalInput")
    # down weights: wd[e, p, o, j, c] = down[e][j*128+p, o*128+c]
    wd_d = nc.dram_tensor("wd", [EXP_PER_CORE, P, DIM // P, ED // P, P], f32r,
                          kind="ExternalInput")
    # shared-expert token shard, transposed like xg
    TS = (4 * 1024) // N_CORES  # 512 tokens per core
    xs_d = nc.dram_tensor("xs", [P, DIM // P, TS], f32r, kind="ExternalInput")
    # shared gate/up: sgu[g_or_u, p, hh, dd, c] = Wsh.T[dd*128+p, hh*128+c]
    sgu_d = nc.dram_tensor("sgu", [2, P, SH // P, DIM // P, P], f32r, kind="ExternalInput")
    # shared down: sd[p, o, j, c] = sh_down.T[j*128+p, o*128+c]
    sd_d = nc.dram_tensor("sd", [P, DIM // P, SH // P, P], f32r, kind="ExternalInput")

    # outputs: ye{s}[o, p, t] = (expert out)[d=o*128+p, token t] * combine
    ye_ds = [nc.dram_tensor(f"ye{i}", [DIM // P, P, CS[i]], f32,
                            kind="ExternalOutput") for i in range(EXP_PER_CORE)]
    ys_d = nc.dram_tensor("ys", [DIM // P, P, TS], f32, kind="ExternalOutput")

    tiless = [_token_tiles(C0), _token_tiles(C1)]
    DD = DIM // P   # 8 feature chunks
    HE = ED // P    # 4 expert-hidden chunks
    HS = SH // P    # 8 shared-hidden chunks

    with tile.TileContext(nc) as tc:
        with (
            tc.tile_pool(name="acts", bufs=1) as acts,
            tc.tile_pool(name="wts", bufs=1) as wts,
            tc.tile_pool(name="outs", bufs=4) as outs,
            tc.tile_pool(name="psum", bufs=3, space="PSUM") as psum,
        ):
            def load_gu_w(n_h, w_dram, tag, nsplit, split_rings=False):
                nb = 4 if tag == "w" else 2
                wg_sb = wts.tile([P, n_h, DD, P], f32r, tag=tag, name="wg", bufs=nb)
                wu_sb = wts.tile([P, n_h, DD, P], f32r, tag=tag, name="wu", bufs=nb)
                step = n_h // nsplit
                ueng = nc.gpsimd if split_rings else nc.sync
                for h0 in range(0, n_h, step):
                    nc.sync.dma_start(wg_sb[:, h0:h0 + step], w_dram[0, :, h0:h0 + step])
                    ueng.dma_start(wu_sb[:, h0:h0 + step], w_dram[1, :, h0:h0 + step])
                return wg_sb, wu_sb

            def gu_phase(n_h, w_tiles, x_sb, hT, toks):
                wg_sb, wu_sb = w_tiles
                for (t0, tsz) in toks:
                    for hh in range(n_h):
                        pg = psum.tile([P, 512], f32, tag="pg", name="pg")[:, :tsz]
                        pu = psum.tile([P, 512], f32, tag="pu", name="pu")[:, :tsz]
                        for d in range(DD):
                            nc.tensor.matmul(pg, wg_sb[:, hh, d, :],
                                             x_sb[:, d, t0:t0 + tsz],
                                             start=(d == 0), stop=(d == DD - 1))
                        for d in range(DD):
                            nc.tensor.matmul(pu, wu_sb[:, hh, d, :],
                                             x_sb[:, d, t0:t0 + tsz],
                                             start=(d == 0), stop=(d == DD - 1))
                        sw = outs.tile([P, 512], f32, tag="sw", name="sw")[:, :tsz]
                        nc.scalar.activation(sw, pg, SIG)  # silu(gate)
                        nc.vector.tensor_mul(hT[:, hh, t0:t0 + tsz], sw, pu)

            def load_down_w(n_h, w_dram, tag):
                wd_sb = wts.tile([P, DD, n_h, P], f32r, tag=tag, name="wd",
                                 bufs=(2 if tag == "wd" else 1))
                for o0 in range(0, DD, DD // 2):
                    nc.sync.dma_start(wd_sb[:, o0:o0 + DD // 2],
                                      w_dram[:, o0:o0 + DD // 2])
                return wd_sb

            def down_phase(n_h, wd_sb, hT, out_d, toks, bc_sb):
                for o in range(DD):
                    for (t0, tsz) in toks:
                        pd = psum.tile([P, 512], f32, tag="pd", name="pd",
                                       bufs=2)[:, :tsz]
                        for j in range(n_h):
                            nc.tensor.matmul(pd, wd_sb[:, o, j, :],
                                             hT[:, j, t0:t0 + tsz],
                                             start=(j == 0), stop=(j == n_h - 1))
                        yt = outs.tile([P, 512], f32, tag="yt", name="yt",
                                       bufs=8)[:, :tsz]
                        if bc_sb is not None:
                            nc.vector.tensor_mul(yt, pd, bc_sb[:, t0:t0 + tsz])
                        else:
                            nc.vector.tensor_copy(yt, pd)
                        nc.sync.dma_start(out_d[o, :, t0:t0 + tsz], yt)

            # activation loads on the (otherwise idle until late) gpsimd queue,
            # in compute-consumption order: shared shard first.
            xs_sb = acts.tile([P, DD, TS], f32r, tag="xs", name="xs")
            for d in range(0, DD, 2):
                nc.gpsimd.dma_start(xs_sb[:, d:d + 2, :], xs_d[:, d:d + 2, :])
            sgu_w = load_gu_w(HS, sgu_d, "sgu", 8, split_rings=True)
            xg_sbs, bc_sbs, hTs = [], [], []
            for e in range(EXP_PER_CORE):
                xg_sb = acts.tile([P, DD, CS[e]], f32r, tag=f"xg{e}", name="xg")
                for d in range(DD):
                    nc.gpsimd.dma_start(xg_sb[:, d, :], xg_ds[e][:, d, :])
                bc_sb = acts.tile([P, CS[e]], f32, tag=f"bc{e}", name="bc")
                nc.gpsimd.dma_start(bc_sb[:], bc_ds[e][:])
                xg_sbs.append(xg_sb)
                bc_sbs.append(bc_sb)
                hTs.append(acts.tile([P, HE, CS[e]], f32r, tag=f"hT{e}", name="hT"))

            # phase order: shared gate/up first (its inputs are small, so it
            # starts earliest), expert gate/ups while their weights stream,
            # shared down mid-kernel, expert downs last (small final outputs).
            sT = acts.tile([P, HS, TS], f32r, tag="sT", name="sT")
            gu_phase(HS, sgu_w, xs_sb, sT, [(0, TS)])
            for e in range(EXP_PER_CORE):
                w = load_gu_w(HE, wgu_d[e], "w", 2)
                gu_phase(HE, w, xg_sbs[e], hTs[e], tiless[e])
            sd_sb = load_down_w(HS, sd_d, "sd")
            wd_sbs = [load_down_w(HE, wd_d[e], "wd") for e in range(EXP_PER_CORE)]
            down_phase(HS, sd_sb, sT, ys_d, [(0, TS)], None)
            for e in range(EXP_PER_CORE):
                down_phase(HE, wd_sbs[e], hTs[e], ye_ds[e], tiless[e],
                           bc_sbs[e])

    nc.compile()
    return nc


def _chunkT(a2d):
    """[K, N] -> [128, K//128, N] with partition = row within 128-chunk."""
    K, N = a2d.shape
    return np.ascontiguousarray(a2d.reshape(K // P, P, N).transpose(1, 0, 2))


def kernel(x, router_w, router_bias, up_proj, gate_proj, down_proj,
           sh_gate, sh_up, sh_down):
    global LAST_RESULT
    from concourse.bass_utils import run_bass_kernel_spmd

    x = np.asarray(x, np.float32)
    B, T, D = x.shape
    N = B * T
    flat = np.ascontiguousarray(x.reshape(N, D))

    # ---- host router (fp64 for a stable top-k; margins >> fp32 noise) ----
    logits = flat.astype(np.float64) @ np.asarray(router_w, np.float64).T \
        + np.asarray(router_bias, np.float64)
    top2 = np.argpartition(-logits, TOPK - 1, axis=1)[:, :TOPK]
    lsel = np.take_along_axis(logits, top2, axis=1)
    lsel -= lsel.max(axis=1, keepdims=True)
    sc = np.exp(lsel)
    sc /= sc.sum(axis=1, keepdims=True)          # [N, 2] combine weights (fp64)

    tok_idx, tok_w = [], []
    for e in range(E):
        rows, slots = np.nonzero(top2 == e)
        tok_idx.append(rows)
        tok_w.append(sc[rows, slots].astype(np.float32))
    cnts = np.array([len(i) for i in tok_idx])
    # load-balance: the 8 busiest experts go to slot 0, the rest to slot 1,
    # so slot 1 gets a smaller capacity (less padded compute).
    order = np.argsort(-cnts, kind="stable")
    slot_experts = [order[:N_CORES], order[N_CORES:]]   # [slot][core] -> expert

    def _cap(mx):
        g = 16 if USE_BF16 else 128
        return max(256, g * math.ceil(mx / g))

    C0 = _cap(max(cnts[e] for e in slot_experts[0]))
    C1 = _cap(max(cnts[e] for e in slot_experts[1]))
    if C1 > C0:
        C0 = C1
    CS = (C0, C1)

    if CS not in _PROGRAMS:
        _PROGRAMS[CS] = _build_program(C0, C1)
    nc = _PROGRAMS[CS]

    # ---- build per-core inputs ----
    flatT = np.ascontiguousarray(flat.T)          # [D, N]
    TS = N // N_CORES

    def gu_pack(w_in_out):                        # [D, H] -> [128, H/128, D/128, 128]
        Din, H = w_in_out.shape
        return np.ascontiguousarray(
            w_in_out.reshape(Din // P, P, H // P, P).transpose(1, 2, 0, 3)
        ).astype(_MM_NP_DT)

    sguT = np.stack([gu_pack(np.asarray(sh_gate, np.float32).T),
                     gu_pack(np.asarray(sh_up, np.float32).T)])
    sdT = gu_pack(np.asarray(sh_down, np.float32).T)

    in_maps = []
    for c in range(N_CORES):
        m = {"xs": np.ascontiguousarray(
            flatT[:, TS * c:TS * (c + 1)].reshape(D // P, P, TS).transpose(1, 0, 2)
        ).astype(_MM_NP_DT), "sgu": sguT, "sd": sdT}
        for j in range(EXP_PER_CORE):
            e = int(slot_experts[j][c])
            Cj = CS[j]
            idx, w = tok_idx[e], tok_w[e]
            xg = np.zeros((P, D // P, Cj), _MM_NP_DT)
            bc = np.zeros((P, Cj), np.float32)
            g = flatT[:, idx]                     # [D, cnt]
            xg[:, :, :len(idx)] = g.reshape(D // P, P, len(idx)).transpose(1, 0, 2).astype(_MM_NP_DT)
            bc[:, :len(idx)] = w[None, :]
            m[f"xg{j}"] = xg
            m[f"bc{j}"] = bc
        m["wgu"] = np.stack([
            np.stack([gu_pack(np.asarray(gate_proj[int(slot_experts[j][c])], np.float32)),
                      gu_pack(np.asarray(up_proj[int(slot_experts[j][c])], np.float32))])
            for j in range(EXP_PER_CORE)])
        m["wd"] = np.stack([gu_pack(np.asarray(down_proj[int(slot_experts[j][c])], np.float32))
                            for j in range(EXP_PER_CORE)])
        in_maps.append(m)

    try:
        res = run_bass_kernel_spmd(nc, in_maps, core_ids=list(range(N_CORES)))
    except Exception:
        res = run_bass_kernel_spmd(nc, in_maps, core_ids=list(range(N_CORES)))
    LAST_RESULT = res

    # ---- unshard: scatter-add expert outputs, add shared shard ----
    y = np.zeros((N, D), np.float32)
    for c in range(N_CORES):
        for j in range(EXP_PER_CORE):
            e = int(slot_experts[j][c])
            idx = tok_idx[e]
            ye = res.results[c][f"ye{j}"]         # [D/128, 128, Cj]
            y[idx] += ye.reshape(D, CS[j])[:, :len(idx)].T
        ys = res.results[c]["ys"].reshape(D, TS)  # [D/128,128,TS] -> [D, TS]
        y[TS * c:TS * (c + 1)] += ys.T
    return y.reshape(B, T, D)

